# revision 59
# baseline (speedup 1.0000x reference)
"""Multi-head attention (D=2048, H=16, B=2, S=2048, causal, RoPE) on 8 TRN2 cores.

Sharding: tensor-parallel over heads -- 2 heads per core, both batches.
Each core computes q/k/v projections for its 2 heads, RoPE, causal flash-style
attention, and a partial output projection over its heads' columns of wo.
The host sums the 8 partial outputs (the out-projection contracts over heads,
which is the sharded axis).

Softmax denominator: below-diagonal attention-weight tiles are pair-summed in
a bf16 tree on the (underutilized) DVE, so the PE only runs one ones-matmul
per tree root plus the (causally masked) diagonal tiles -- ~2.6x less PE time
than a per-k-tile ones-matmul. Output partials are DMA'd as bf16 (halves the
output traffic; the host accumulates in f32).

Self-contained: hardcodes all shapes; only needs numpy/ml_dtypes/concourse.
"""
import os
import sys
import time

for _p in ("/opt/trn_rl_repo",):
    if os.path.isdir(_p) and _p not in sys.path:
        sys.path.append(_p)

import numpy as np
import ml_dtypes
from contextlib import ExitStack

import concourse.bass as bass
import concourse.tile as tile
from concourse import bacc, mybir

BF = mybir.dt.bfloat16
F32 = mybir.dt.float32
BF_NP = ml_dtypes.bfloat16

B = 2
S = 2048
D = 2048
H = 16
HD = 128  # head dim
N_CORES = 8
H_CORE = H // N_CORES          # heads per core = 2
E = H_CORE * HD                # per-core q/k/v width = 256
BS = B * S                     # 4096 flattened tokens
P = 128
SC = 512                       # s-chunk (free dim of projection matmuls)
N_SC = BS // SC                # 8 s-chunks
N_DT = D // P                  # 16 d-tiles (contraction)
QC = 512                       # q-chunk in attention
N_QC = S // QC                 # 4 q-chunks per (batch, head)
N_KT = S // P                  # 16 k-tiles per (batch, head)
SCALE = 1.0 / float(np.sqrt(HD))
ROPE_BASE = 10000.0


def _build_program():
    """Build the per-core Bass program (identical on all cores; data differs)."""
    nc = bacc.Bacc("TRN2", target_bir_lowering=False, debug=False)

    # all big inputs are host-packed to the exact SBUF layout so every DMA is
    # one long contiguous run per partition (few descriptors, fast HWDGE)
    xt_d = nc.dram_tensor("xt", [N_SC, P, N_DT * SC], BF, kind="ExternalInput").ap()
    wqt_d = nc.dram_tensor("wqt", [P, N_DT * E], BF, kind="ExternalInput").ap()
    wkt_d = nc.dram_tensor("wkt", [P, N_DT * E], BF, kind="ExternalInput").ap()
    wvt_d = nc.dram_tensor("wvt", [P, N_DT * E], BF, kind="ExternalInput").ap()
    wot_d = nc.dram_tensor("wot", [P, H_CORE * D], BF, kind="ExternalInput").ap()
    cos_d = nc.dram_tensor("cos", [P, S], BF, kind="ExternalInput").ap()
    sin_d = nc.dram_tensor("sin", [P, S], BF, kind="ExternalInput").ap()
    rmat_d = nc.dram_tensor("rmat", [P, P], BF, kind="ExternalInput").ap()
    tri_d = nc.dram_tensor("tri", [P, P], BF, kind="ExternalInput").ap()
    out_d = nc.dram_tensor("out", [BS, D], BF, kind="ExternalOutput").ap()

    with tile.TileContext(nc) as tc:
        with ExitStack() as ctx:
            _emit(ctx, tc, nc, xt_d, wqt_d, wkt_d, wvt_d, wot_d,
                  cos_d, sin_d, rmat_d, tri_d, out_d)
    nc.compile()
    return nc


def _emit(ctx, tc, nc, xt_d, wqt_d, wkt_d, wvt_d, wot_d,
          cos_d, sin_d, rmat_d, tri_d, out_d):
    Exp = mybir.ActivationFunctionType.Exp

    const = ctx.enter_context(tc.tile_pool(name="const", bufs=1))
    xpool = ctx.enter_context(tc.tile_pool(name="xpool", bufs=int(os.environ.get("KXP","2"))))
    qkv = ctx.enter_context(tc.tile_pool(name="qkv", bufs=1))
    rope = ctx.enter_context(tc.tile_pool(name="rope", bufs=int(os.environ.get("KROPE","4"))))
    att = ctx.enter_context(tc.tile_pool(name="att", bufs=8))
    tsum = ctx.enter_context(tc.tile_pool(name="tsum", bufs=int(os.environ.get("KTS","6"))))
    nrm = ctx.enter_context(tc.tile_pool(name="nrm", bufs=int(os.environ.get("KNRM","4"))))
    outp = ctx.enter_context(tc.tile_pool(name="outp", bufs=int(os.environ.get("KOUTP","8"))))
    psum = ctx.enter_context(tc.tile_pool(name="psum", bufs=int(os.environ.get("KACC","5")), space="PSUM"))
    psum_s = ctx.enter_context(tc.tile_pool(name="psum_s", bufs=int(os.environ.get("KSTR","3")), space="PSUM"))

    # ---- constants / weights in SBUF ----
    def load_xt_chunk(sc, eng):
        xt_c = xpool.tile([P, N_DT * SC], BF, tag="xt")
        eng.dma_start(xt_c[:], xt_d[sc])  # 16 KB contiguous per partition
        return xt_c

    # DMA ordering for the startup ramp (2 HWDGE queues: SP=sync, ACT=scalar).
    # Use order: first-needed data first, split into pieces so the first
    # accumulation group can start as soon as its first d-tiles arrive; wk
    # pieces are interleaved right behind wq so the k-projection is never
    # starved.
    xt_c0 = xpool.tile([P, N_DT * SC], BF, tag="xt")
    wq_sb = const.tile([P, N_DT * E], BF)
    wk_sb = const.tile([P, N_DT * E], BF)
    wv_sb = const.tile([P, N_DT * E], BF)
    # priority: x0 > wq > wk > x1/x2 (queued right behind by emit_phase1)
    # > wv > cos/sin. RoPE (cos/sin) can lag: qT/kT aren't consumed until
    # attention starts ~100us later, and a late t1/t2 only delays the
    # slack-rich GpSimd/DVE queues.
    # sync-queue order matches the split-contraction consumption order of
    # chunk 0: x0[0:8] (q first halves), wk[0:8] (k first halves), x0[8:16]
    # (q second halves), wk[8:16] (k second halves) -- each transfer lands
    # just ahead of its consumer pass
    xt_pieces = [(0, 4), (4, 8), (8, 16)]
    for t0_, t1_ in xt_pieces:
        if t1_ == 16:
            nc.sync.dma_start(wk_sb[:, :N_DT // 2 * E],
                              wkt_d[:, :N_DT // 2 * E])
        nc.sync.dma_start(xt_c0[:, t0_ * SC:t1_ * SC],
                          xt_d[0][:, t0_ * SC:t1_ * SC])
        nc.scalar.dma_start(wq_sb[:, t0_ * E:t1_ * E],
                            wqt_d[:, t0_ * E:t1_ * E])
    nc.sync.dma_start(wk_sb[:, N_DT // 2 * E:], wkt_d[:, N_DT // 2 * E:])
    nc.scalar.dma_start(wv_sb[:], wvt_d[:])
    rmat_sb = const.tile([P, P], BF)
    tri_sb = const.tile([P, P], BF)
    nc.scalar.dma_start(rmat_sb[:], rmat_d[:])
    nc.scalar.dma_start(tri_sb[:], tri_d[:])
    cos_sb = const.tile([P, S], BF)
    sin_sb = const.tile([P, S], BF)
    nc.scalar.dma_start(cos_sb[:], cos_d[:])
    nc.scalar.dma_start(sin_sb[:], sin_d[:])
    ones_sb = const.tile([P, P], BF)
    nc.vector.memset(ones_sb[:], 1.0)
    # wot in [128, 2 * D] packed layout; needed only in phase 3 so loaded last
    wo_sb = const.tile([P, H_CORE * D], BF)
    nc.scalar.dma_start(wo_sb[:], wot_d[:])

    # persistent activations
    qT = qkv.tile([P, H_CORE * BS], BF)   # [d, (head, b*s)] rope'd q
    kT = qkv.tile([P, H_CORE * BS], BF)   # [d, (head, b*s)] rope'd k
    v_sb = qkv.tile([P, (BS // P) * E], BF)  # [s within tile, (s-tile, e)]
    aoT = qkv.tile([P, H_CORE * BS], BF)  # [d, (b, head, q)] normalized attn out

    # ---- phase 1: projections + RoPE ----
    def do_rope(raw, s_lo, dst):
        rot = psum_s.tile([P, SC], F32, tag="pss")
        nc.tensor.matmul(rot[:], rmat_sb[:], raw[:], start=True, stop=True)
        t1 = rope.tile([P, SC], BF, tag="t1")
        # raw * cos is SBUF-only: run it on the otherwise-idle GpSimd
        nc.gpsimd.tensor_mul(t1[:], raw[:], cos_sb[:, s_lo:s_lo + SC])
        t2 = rope.tile([P, SC], BF, tag="t2")
        nc.vector.tensor_mul(t2[:], rot[:], sin_sb[:, s_lo:s_lo + SC])
        nc.vector.tensor_add(dst, t1[:], t2[:])

    def emit_phase1(sc, defer_v=False):
        # defer_v: return the two v-projection groups as closures instead of
        # emitting them, so the drive can weave them into attention k-loops
        # as PE filler work (batch-1 v isn't consumed for another ~100us)
        b = sc // (N_SC // B)
        s_lo = (sc % (N_SC // B)) * SC  # within-batch s offset
        if sc == 0:
            xt_c = xt_c0
        else:
            xt_c = load_xt_chunk(sc, nc.sync)

        # qT / kT (with RoPE) per head (e-tile == head). The rot matmul of
        # each head is deferred until after the next head's projection chain:
        # the in-order PE would otherwise sit at the rot waiting for the ACT
        # raw-copy instead of streaming the next chain.
        def qk_part(w_sb, h, pp, t0, t1, first, last):
            for t in range(t0, t1):
                nc.tensor.matmul(
                    pp[:],
                    w_sb[:, t * E + h * HD: t * E + h * HD + HD],
                    xt_c[:, t * SC:(t + 1) * SC],
                    start=(first and t == t0), stop=(last and t == t1 - 1))

        pend_rope = None
        units = [(w_sb, dstT, h) for w_sb, dstT in ((wq_sb, qT), (wk_sb, kT))
                 for h in range(H_CORE)]
        if sc == 0:
            # chunk 0: the first pass over x0+wq consumes 3 MB faster than the
            # two DMA queues can feed it. Run all four chains' first-half
            # contractions (tiles 0-7) before any second half, so early PE
            # consumption matches DMA delivery and the ramp never starves.
            pps = [psum.tile([P, SC], F32, tag="ps", name=f"pp{i}")
                   for i in range(len(units))]
            for (w_sb, dstT, h), pp in zip(units, pps):
                qk_part(w_sb, h, pp, 0, N_DT // 2, True, False)
            for (w_sb, dstT, h), pp in zip(units, pps):
                qk_part(w_sb, h, pp, N_DT // 2, N_DT, False, True)
                raw = rope.tile([P, SC], BF, tag="raw")
                nc.scalar.copy(raw[:], pp[:])
                if pend_rope is not None:
                    do_rope(*pend_rope)
                dst = dstT[:, h * BS + sc * SC: h * BS + (sc + 1) * SC]
                pend_rope = (raw, s_lo, dst)
        else:
            for w_sb, dstT, h in units:
                pp = psum.tile([P, SC], F32, tag="ps")
                qk_part(w_sb, h, pp, 0, N_DT, True, True)
                raw = rope.tile([P, SC], BF, tag="raw")
                nc.scalar.copy(raw[:], pp[:])
                if pend_rope is not None:
                    do_rope(*pend_rope)
                dst = dstT[:, h * BS + sc * SC: h * BS + (sc + 1) * SC]
                pend_rope = (raw, s_lo, dst)

        # v for this s-chunk: 4 s-subtiles of 128, two per PSUM tile so each
        # eviction copy covers 512 columns
        def v_group(sp, flush_rope):
            pv = psum.tile([P, SC], F32, tag="ps")
            for half in range(2):
                st = sp * 2 + half
                for t in range(N_DT):
                    nc.tensor.matmul(
                        pv[:, half * E:(half + 1) * E],
                        xt_c[:, t * SC + st * P: t * SC + (st + 1) * P],
                        wv_sb[:, t * E:(t + 1) * E],
                        start=(t == 0), stop=(t == N_DT - 1))
            if flush_rope is not None:
                do_rope(*flush_rope)
            g_st = sc * (SC // P) + sp * 2  # global s-tile index
            nc.scalar.copy(v_sb[:, g_st * E:(g_st + 2) * E], pv[:])

        groups = [lambda sp=sp, fr=(pend_rope if sp == 0 else None):
                  v_group(sp, fr) for sp in range(SC // P // 2)]
        pend_rope = None
        if defer_v:
            return groups
        for g in groups:
            g()

    # ---- phase 2 + 3: attention per (batch, head); out-projection for each
    # q-chunk emitted as soon as both heads' attention output is ready, so the
    # output DMA overlaps the remaining attention compute ----
    def attention_chunk(b, h, qc, fillers=()):
        # fillers: closures emitted at evenly spaced points of the k-loop
        # (used to spread the previous chunk's out-projection so its PSUM use
        # and eviction load drain gradually instead of in one burst)
        fillers = list(fillers)
        qk_off = h * BS + b * S  # column offset into qT/kT
        out_ps = psum.tile([P, QC], F32, tag="ps")
        den_ps = psum.tile([P, QC], F32, tag="ps")
        nkt = (qc + 1) * (QC // P)
        ndiag = qc * (QC // P)  # number of full (below-diagonal) k-tiles
        # front-load fillers: the first runs at step 0, where the PE would
        # otherwise idle waiting for the first exp of the chunk
        fill_at = {(i * nkt) // len(fillers): f
                   for i, f in enumerate(fillers)} if fillers else {}
        # bf16 pairwise-summation tree over the below-diagonal at tiles (DVE);
        # the PE then reduces each root with a single ones-matmul into den_ps.
        # Eager combine keeps <= log2 levels alive.
        tstack = []  # (level, tile)

        def tree_push(node):
            level = 0
            while tstack and tstack[-1][0] == level:
                _, prev = tstack.pop()
                merged = tsum.tile([P, QC], BF, tag="ts")
                nc.vector.tensor_add(merged[:], prev[:], node[:])
                node = merged
                level += 1
            tstack.append((level, node))

        den_started = [False]

        def den_matmul(src_ap, q0, stop):
            nc.tensor.matmul(
                den_ps[:, q0:QC], ones_sb[:], src_ap,
                start=not den_started[0], stop=stop)
            den_started[0] = True

        def consume(j, at, q0, di):
            # AV + denominator for tile j; emitted one step behind the QK/exp
            # so the in-order PE never has an at-dependent matmul at its queue
            # head while the next QK (or a filler) could run
            nc.tensor.matmul(
                out_ps[:, q0:QC],
                v_sb[:, (b * (S // P) + j) * E + h * HD:
                      (b * (S // P) + j) * E + (h + 1) * HD],
                at[:, q0:],
                start=(j == 0), stop=(j == nkt - 1))
            if di < 0:
                if j == ndiag - 1:
                    # reduce all tree roots into den_ps (deepest first)
                    for _, node in tstack:
                        den_matmul(node[:], 0, stop=False)
                    tstack.clear()
            else:
                den_matmul(at[:, q0:], q0, stop=(j == nkt - 1))

        pend = []
        for j in range(nkt):
            at = att.tile([P, QC], BF, tag="at",
                          bufs=int(os.environ.get("KAT", "5")))
            di = j - ndiag
            q0 = max(di, 0) * P  # valid q suffix start
            sc_ps = psum_s.tile([P, QC], F32, tag="pss")
            nc.tensor.matmul(
                sc_ps[:, q0:],
                kT[:, qk_off + j * P: qk_off + (j + 1) * P],
                qT[:, qk_off + qc * QC + q0: qk_off + (qc + 1) * QC],
                start=True, stop=True)
            nc.scalar.activation(at[:, q0:], sc_ps[:, q0:], Exp, scale=SCALE)
            if di >= 0:
                nc.vector.tensor_mul(at[:, q0:q0 + P],
                                     at[:, q0:q0 + P], tri_sb[:])
            elif di < 0:
                tree_push(at)
            if j in fill_at:
                fill_at[j]()
            pend.append((j, at, q0, di))
            if len(pend) > int(os.environ.get("KDEPTH", "1")):
                consume(*pend.pop(0))
        for p in pend:
            consume(*p)
        rec = nrm.tile([P, QC], F32, tag="rec")
        nc.vector.reciprocal_approx_fast(rec[:], den_ps[:])
        dst = aoT[:, (b * H_CORE + h) * S + qc * QC:
                  (b * H_CORE + h) * S + (qc + 1) * QC]
        nc.vector.tensor_mul(dst, out_ps[:], rec[:])

    def outproj_st(b, st, tail=False):
        # one 128-row slab of batch b's output, all 4 e-chunks
        for ec in range(D // SC):
            po = psum.tile([P, SC], F32, tag="ps")
            for h in range(H_CORE):
                lhsT = aoT[:, (b * H_CORE + h) * S + st * P:
                           (b * H_CORE + h) * S + (st + 1) * P]
                nc.tensor.matmul(
                    po[:],
                    lhsT,
                    wo_sb[:, h * D + ec * SC: h * D + (ec + 1) * SC],
                    start=(h == 0), stop=(h == H_CORE - 1))
            o_sb = outp.tile([P, SC], BF, tag="o")
            if tail and ec % 2 == 0:
                # at the tail ACT is idle; otherwise keep ACT exp-only so
                # evictions never delay the exp critical path
                nc.scalar.copy(o_sb[:], po[:])
            else:
                nc.vector.tensor_copy(o_sb[:], po[:])
            dma_eng = nc.scalar if (tail and ec % 2 == 1) else nc.sync
            dma_eng.dma_start(
                out_d[b * S + st * P: b * S + (st + 1) * P,
                      ec * SC:(ec + 1) * SC],
                o_sb[:])

    def outproj_fillers(b, qc, tail=False):
        return [lambda st=st: outproj_st(b, st, tail)
                for st in range(qc * (QC // P), (qc + 1) * (QC // P))]

    def drive(emit_phase1):
        if os.environ.get("KILV", "1") == "1":
            # batch 0 projections first, then batch 1 projections interleaved
            # with batch 0 attention (spreads ACT/DVE-heavy attention work
            # into the ACT-light projection region)
            for sc in range(N_SC // B):
                emit_phase1(sc)
            for qc in range(N_QC):
                vg = emit_phase1(N_SC // B + qc, defer_v=True)
                prev = (0, qc - 1)
                fa = outproj_fillers(*prev) if prev[1] >= 0 else []
                # split the previous chunk's out-projection slabs and this
                # phase-1 chunk's deferred v-groups across both heads' chunks
                # so each chunk start has PE filler work
                attention_chunk(0, 0, qc, fillers=fa[:2] + vg[:1])
                attention_chunk(0, 1, qc, fillers=fa[2:] + vg[1:])
            for qc in range(N_QC):
                prev = (1, qc - 1) if qc > 0 else (0, N_QC - 1)
                fa = outproj_fillers(*prev)
                attention_chunk(1, 0, qc, fillers=fa[:2])
                attention_chunk(1, 1, qc, fillers=fa[2:])
        else:
            for sc in range(N_SC):
                emit_phase1(sc)
            for b in range(B):
                for qc in range(N_QC):
                    attention_chunk(b, 0, qc)
                    prev = (b, qc - 1) if qc > 0 else (b - 1, N_QC - 1)
                    fillers = outproj_fillers(*prev) if prev[0] >= 0 else ()
                    attention_chunk(b, 1, qc, fillers=fillers)
        for f in outproj_fillers(B - 1, N_QC - 1, tail=True):
            f()
    drive(emit_phase1)


def _rope_tables():
    """cos/sin tables exactly matching the reference's indexing quirk."""
    inv_freq = (1.0 / (ROPE_BASE ** (np.arange(0, HD, 2, dtype=np.float32) / HD)))
    t = np.arange(S, dtype=np.float32)
    freqs = np.outer(t, inv_freq)                       # [S, 64]
    emb = np.concatenate([freqs, freqs], axis=1)        # [S, 128]
    cos_part = np.cos(emb)[:, ::2]                      # [S, 64]
    sin_part = np.sin(emb)[:, 1::2]                     # [S, 64]
    # COS[d, s] = cos_part[s, d // 2]
    cos = cos_part.T[np.repeat(np.arange(HD // 2), 2)]  # [128, S]
    sin = sin_part.T[np.repeat(np.arange(HD // 2), 2)]
    return np.ascontiguousarray(cos), np.ascontiguousarray(sin)


def _pack_dtile_major(wt):
    """[D, E] (d, e) -> [128, N_DT * E]: row p holds [t, e] contiguously."""
    d, e = wt.shape
    return np.ascontiguousarray(
        wt.reshape(d // P, P, e).transpose(1, 0, 2).reshape(P, (d // P) * e))


def _host_prep(x, wq, wk, wv, wo):
    """Build the per-core input maps (SBUF-layout packed, bf16)."""
    bf = BF_NP
    xt = x.reshape(BS, D).T.astype(bf)                  # [D, BS]
    # pack to [N_SC, 128, N_DT*SC]: chunk sc, partition p -> (t, s) contiguous
    xt = np.ascontiguousarray(
        xt.reshape(N_DT, P, N_SC, SC).transpose(2, 1, 0, 3).reshape(
            N_SC, P, N_DT * SC))
    cos, sin = _rope_tables()
    cos = cos.astype(bf)
    sin = sin.astype(bf)
    rmat = np.zeros((P, P), dtype=np.float32)           # R^T for rot = R @ q
    idx = np.arange(0, P, 2)
    rmat[idx + 1, idx] = -1.0                           # R^T[2j+1, 2j] = -1
    rmat[idx, idx + 1] = 1.0                            # R^T[2j, 2j+1] = +1
    rmat = rmat.astype(bf)
    tri = np.triu(np.ones((P, P), dtype=np.float32)).astype(bf)

    in_maps = []
    for c in range(N_CORES):
        lo, hi = c * E, (c + 1) * E
        in_maps.append({
            "xt": xt,
            "wqt": _pack_dtile_major(wq[lo:hi].T.astype(bf)),
            "wkt": _pack_dtile_major(wk[lo:hi].T.astype(bf)),
            "wvt": _pack_dtile_major(wv[lo:hi].T.astype(bf)),
            "wot": _pack_dtile_major(wo[:, lo:hi].T.astype(bf)),
            "cos": cos,
            "sin": sin,
            "rmat": rmat,
            "tri": tri,
        })
    return in_maps


_CACHE = {}


def _get_program():
    if "nc" not in _CACHE:
        _CACHE["nc"] = _build_program()
    return _CACHE["nc"]


def _run(in_maps):
    from concourse.bass_utils import run_bass_kernel_spmd
    nc = _get_program()
    res = run_bass_kernel_spmd(nc, in_maps, core_ids=list(range(N_CORES)))
    return res


def kernel(x, wq, wk, wv, wo, attn_mask=None, **_):
    x = np.asarray(x, dtype=np.float32)
    in_maps = _host_prep(np.asarray(x, np.float32), np.asarray(wq, np.float32),
                         np.asarray(wk, np.float32), np.asarray(wv, np.float32),
                         np.asarray(wo, np.float32))
    res = _run(in_maps)
    out = np.zeros((BS, D), dtype=np.float32)
    for c in range(N_CORES):
        out += res.results[c]["out"].astype(np.float32)
    return out.reshape(B, S, D)


if __name__ == "__main__":
    t0 = time.time()
    _get_program()
    print(f"program build: {time.time() - t0:.1f}s")
